# revision 31
# baseline (speedup 1.0000x reference)
"""Trainium2 Bass kernel for nn_Attention (dense transformer block), v4.

v4 = v3 + structural pipeline work:
  - NOTE: fp8 for the v projection was tried and measured +3e-2 error: the
    projection output is itself a random-sign sum, so input quantization
    error does NOT average down over the contraction (error scales with
    sqrt(sum of squares) = the same sqrt(K) as the output). v stays bf16.
  - initial DMAs are split per feature plane / per 128-row x slice and
    ordered by need (wq/x8/wk planes feed the t-th matmul of the first q/k
    projection; xT slice jt gates pv(jt) alone), so the first matmul starts
    ~1.5 us after kernel start and attention chunk (0,0) by ~7 us.
  - normalize (recip+mul) is emitted per column block gated on the diagonal
    AV that finalizes that block, instead of one per-chunk burst: mid chunks
    use 256-col blocks, the final chunk 128-col blocks so the tail
    interleaves output-projection quanta with the last normalizes.
  - background matmul drain is deadline-driven with even smoothing over the
    remaining global tile count (v3's per-chunk formula front-loaded the
    drain and starved the last ~20 tiles, leaving ~300 ns/tile PE gaps).
  - initial DMAs are split per feature plane and spread across the SP and
    DVE queues in need order, so the first q/k projection matmul starts
    ~1.5 us after kernel start instead of ~4.6 us.
  - output-projection PSUM->SBUF copies alternate DVE/Pool mid-kernel and
    ACT/Pool in the tail (ACT is idle once the exp stream ends).

Reference computation (per batch b of 2, seq N=2048, dim D=1024, 16 heads x 64):
    q = (x @ w_q) / 64                      # source double-scales by d**-0.5
    k, v = split(x @ w_kv)
    per head: out_h = softmax(causal(q_h k_h^T)) v_h
    y = concat(out) @ w_out + b_out

Sharding (8 cores): core c -> batch b = c//4, head group g = c%4 (heads 4g..4g+3).
Each core computes its 4 heads end-to-end plus its partial output projection
(rows 256g..256g+256 of w_out); the host sums the 4 partials per batch and adds
b_out.

Carried from v3/v2:
  - fp8e4 DoubleRow for q/k/v projections (K=256/matmul); attention QK^T/AV
    and the output projection stay bf16 (fp8 there measured +2-3e-2 error).
  - AV lhsT per head is [v_h | c*ones] or [c*ones | v_h] (128 cols), so the
    softmax denominator comes out REPLICATED on 64 PSUM partitions; the
    normalization is DVE reciprocal + multiply with no PE broadcast.
  - the two heads' QK^T matmuls sit at lhsT base partitions 0/64 -> walrus
    derives row-group tile_position, so they overlap on the PE array.
  - engine assignment: ACT does exp; DVE does PSUM->SBUF copies + normalize;
    GPSIMD (Pool) does causal triangle masks, ones memset, some copies.
"""

import numpy as np

import concourse.bass as bass
import concourse.mybir as mybir
import concourse.tile as tile
from concourse import bacc
from concourse.bass_utils import run_bass_kernel_spmd

FP = mybir.dt.float32
BF = mybir.dt.bfloat16
F8 = mybir.dt.float8e4
DR = mybir.MatmulPerfMode.DoubleRow
EXP = mybir.ActivationFunctionType.Exp
WS = 32.0  # host prescale on w_q/w_k/w_v (exact power of 2)

B = 2
N = 2048  # sequence length
D = 1024  # model dim
NH = 4  # heads per core
DH = 64  # head dim
G = NH * DH  # 256 = per-core projection width
P = 128
DKT = D // P  # 8 feature k-tiles
PT2 = DKT // 2  # 4 paired fp8 feature planes
KT = N // P  # 16 sequence k-tiles
QC = N // 512  # 4 q chunks of 512
NCORES = 8

# global tile index at which chunk (pr, c) starts; chunk order is
# (0,0),(1,0),(0,1),(1,1),(0,2),(1,2),(0,3),(1,3)
CHUNK_START = {}
_t = 0
for _c in range(QC):
    for _pr in range(2):
        CHUNK_START[(_pr, _c)] = _t
        _t += 4 * _c + 4
TOTAL_TILES = _t  # 80


def build_bass(repeat=1, hw_loop=0, upto="full", loads_once=False):
    nc = bacc.Bacc("TRN2", target_bir_lowering=False, debug=False, num_devices=NCORES)

    xT = nc.dram_tensor("xT", [D, N], BF, kind="ExternalInput").ap()
    xT8 = nc.dram_tensor("xT8", [D, N], F8, kind="ExternalInput").ap()
    wq = nc.dram_tensor("wq", [D, G], F8, kind="ExternalInput").ap()
    wk = nc.dram_tensor("wk", [D, G], F8, kind="ExternalInput").ap()
    wv = nc.dram_tensor("wv", [D, G], BF, kind="ExternalInput").ap()  # cols h0,h2,h1,h3
    wo = nc.dram_tensor("wo", [G, D], BF, kind="ExternalInput").ap()
    tri = nc.dram_tensor("tri", [P, P], BF, kind="ExternalInput").ap()
    y = nc.dram_tensor("y", [N, D], FP, kind="ExternalOutput").ap()

    with tile.TileContext(nc) as tc:
        with (
            tc.tile_pool(name="const", bufs=1) as const,
            tc.tile_pool(name="ptp", bufs=6) as ptp,
            tc.tile_pool(name="ysb", bufs=4) as ysbp,
            tc.tile_pool(name="rsp", bufs=2) as rsp,
            tc.tile_pool(name="psum", bufs=2, space="PSUM") as psum,
        ):
            # fp8 paired layouts [P, t, ko, ...]: feature plane = 2t+ko
            wq_sb = const.tile([P, PT2, 2, G], F8)
            wk_sb = const.tile([P, PT2, 2, G], F8)
            xT8_sb = const.tile([P, PT2, 2, N], F8)
            wv_sb = const.tile([P, DKT, G], BF)
            xT_sb = const.tile([P, DKT, N], BF)
            wo_sb = const.tile([P, 2, D], BF)
            tri_sb = const.tile([P, P], BF)
            qT_sb = const.tile([P, 2, N], BF)
            kT_sb = const.tile([P, 2, N], BF)
            # v planes per (jtile, pr): [v_{2pr} | ones | ones | v_{2pr+1}]
            v_sb = const.tile([P, KT, 2, 256], BF)
            oT_sb = const.tile([P, 2, N], BF)

            xTr = xT.rearrange("(o p) m -> p o m", p=P)
            xT8r = xT8.rearrange("(t k p) n -> p t k n", p=P, k=2)
            wqr = wq.rearrange("(t k p) m -> p t k m", p=P, k=2)
            wkr = wk.rearrange("(t k p) m -> p t k m", p=P, k=2)

            def load_initial():
                # need order: plane t of {wq, x8 chunk0, wk} feeds the t-th
                # matmul of the first q/k projections; xT tile slices jt gate
                # pv(jt) individually. wv/tri ride the ACT queue (one issue
                # each, before the exp stream starts).
                # two issue queues in parallel: SP carries the q-side
                # criticals + x slices, ACT carries the k-side + wv/tri
                # (both are needed at the same time; issue overhead halves)
                for t in range(PT2):
                    nc.sync.dma_start(wq_sb[:, t], wqr[:, t])
                    nc.sync.dma_start(
                        xT8_sb[:, t, :, 0:512], xT8r[:, t, :, 0:512]
                    )
                    nc.scalar.dma_start(wk_sb[:, t], wkr[:, t])
                nc.scalar.dma_start(wv_sb, wv.rearrange("(o p) m -> p o m", p=P))
                nc.scalar.dma_start(tri_sb, tri)
                for jt in range(4):
                    nc.sync.dma_start(
                        xT_sb[:, :, P * jt : P * (jt + 1)],
                        xTr[:, :, P * jt : P * (jt + 1)],
                    )

            def load_rest():
                def x8_ch(ch):
                    nc.sync.dma_start(
                        xT8_sb[:, :, :, 512 * ch : 512 * (ch + 1)],
                        xT8r[:, :, :, 512 * ch : 512 * (ch + 1)],
                    )

                def xT_ch(ch):
                    nc.sync.dma_start(
                        xT_sb[:, :, 512 * ch : 512 * (ch + 1)],
                        xTr[:, :, 512 * ch : 512 * (ch + 1)],
                    )

                x8_ch(1)
                xT_ch(1)
                nc.sync.dma_start(wo_sb, wo.rearrange("(o p) m -> p o m", p=P))
                x8_ch(2)
                xT_ch(2)
                x8_ch(3)
                xT_ch(3)

            def ones_memset():
                nc.gpsimd.memset(v_sb[:, :, :, 64:192], 1.0)

            # ---- background quanta -------------------------------------
            def pq(w_sb, dst, pl, half):
                # dst[:, pl, 512*half:+512] = (w plane pl)^T @ xT cols
                def mms():
                    ps = psum.tile([P, 512], FP, tag="rb", name=f"ps_p{pl}h{half}")
                    lo = 512 * half
                    for t in range(PT2):
                        yield nc.tensor.matmul(
                            ps,
                            w_sb[:, t, :, P * pl : P * (pl + 1)],
                            xT8_sb[:, t, :, lo : lo + 512],
                            start=(t == 0),
                            stop=(t == PT2 - 1),
                            perf_mode=DR,
                        )
                    nc.vector.tensor_copy(dst[:, pl, lo : lo + 512], ps)

                return mms, PT2

            def pv(jt):
                # v rows 128*jt..+128, all 4 heads; wv host col order h0,h2,h1,h3
                def mms():
                    ps = psum.tile([P, G], FP, tag="rb", name="ps_v")
                    for j in range(DKT):
                        yield nc.tensor.matmul(
                            ps,
                            xT_sb[:, j, P * jt : P * (jt + 1)],
                            wv_sb[:, j, :],
                            start=(j == 0),
                            stop=(j == DKT - 1),
                        )
                    # one 4D copy: dst (pr, side) cols {0:64, 192:256},
                    # src head index a = pr + 2*side (host col order h0,h2,h1,h3)
                    psv = ps.rearrange("p (s pr d) -> p pr s d", s=2, pr=2)
                    dst = v_sb[:, jt].rearrange("p pr (s d) -> p pr s d", s=4)
                    nc.vector.tensor_copy(dst[:, :, 0:4:3, :], psv)

                return mms, DKT

            def op(i, tail=False):
                # y rows 128*i..+128
                def mms():
                    ysb = ysbp.tile([P, D], FP, name="ysb")
                    for n2 in range(2):
                        ps = psum.tile([P, 512], FP, tag="rb", name="ps_y")
                        for pr in range(2):
                            yield nc.tensor.matmul(
                                ps,
                                oT_sb[:, pr, P * i : P * (i + 1)],
                                wo_sb[:, pr, 512 * n2 : 512 * (n2 + 1)],
                                start=(pr == 0),
                                stop=(pr == 1),
                            )
                        dst = ysb[:, 512 * n2 : 512 * (n2 + 1)]
                        if tail and n2 == 0:  # ACT is idle once the exps end
                            nc.scalar.copy(dst, ps)
                        else:  # Pool can't read PSUM, so DVE otherwise
                            nc.vector.tensor_copy(dst, ps)
                        dmae = nc.scalar if tail and n2 == 1 else nc.sync
                        dmae.dma_start(
                            y[P * i : P * (i + 1), 512 * n2 : 512 * (n2 + 1)],
                            ysb[:, 512 * n2 : 512 * (n2 + 1)],
                        )

                return mms, 4

            class BG:
                """Deadline-driven drain of background matmul generators.

                Items carry (generator, mms_left, global-tile deadline or
                None, earliest-start tile). Per tile: drain everything
                overdue, plus an even smoothing share of the remaining work
                over the remaining tiles — but never an item before its
                earliest-start tile (its input DMA hasn't landed; emitting
                it would stall the in-order PE queue on the DMA).
                """

                def __init__(self):
                    self.items = []  # [gen, mms_left, deadline, est]
                    self.t_now = 0

                def add(self, fac_mms, deadline=None, est=0):
                    fac, mms = fac_mms
                    self.items.append([fac(), mms, deadline, est])

                def _drain_one(self):
                    # among eligible (est <= t_now): items due within 4 tiles
                    # get EDF priority; otherwise FIFO by add order (so
                    # far-deadline items don't starve deadline-less op work)
                    best = None
                    for it in self.items:
                        if it[3] > self.t_now:
                            continue
                        if it[2] is not None and it[2] <= self.t_now + 4 and (
                            best is None or best[2] is None or it[2] < best[2]
                        ):
                            best = it
                    if best is None:
                        for it in self.items:
                            if it[3] <= self.t_now:
                                best = it
                                break
                    if best is None:
                        return False
                    try:
                        next(best[0])
                        best[1] -= 1
                        if best[1] <= 0:
                            # let the generator run its epilogue (copies)
                            try:
                                next(best[0])
                            except StopIteration:
                                pass
                            self.items.remove(best)
                    except StopIteration:
                        self.items.remove(best)
                    return True

                def drain_n(self, n):
                    for _ in range(n):
                        if not self._drain_one():
                            return

                def drain_for_tile(self, t_global):
                    self.t_now = t_global
                    total = sum(it[1] for it in self.items)
                    if not total:
                        return
                    overdue = sum(
                        it[1]
                        for it in self.items
                        if it[2] is not None and it[2] <= t_global
                    )
                    remaining_tiles = max(1, TOTAL_TILES - t_global)
                    smooth = -(-total // remaining_tiles)  # ceil
                    self.drain_n(max(overdue, smooth))

                def drain_all(self):
                    self.t_now = 1 << 30
                    while self._drain_one():
                        pass

            # ---- attention ---------------------------------------------
            def attn_chunk(bg, pr, c, norm_grain=256, post_block=None, av_lag=1):
                # heads (2*pr, 2*pr+1); q columns 512*c..+512. av_lag defers
                # each AV by that many tiles behind its exp (PT pool holds 4),
                # decoupling the first chunk from the v-tile DMA stream.
                outs = [
                    psum.tile([P, 512], FP, tag="o", name=f"o{h2}") for h2 in range(2)
                ]
                rsb = rsp.tile([P, 512], FP, name="rsb")
                last = 4 * c + 3
                t0 = CHUNK_START[(pr, c)]
                pend = []

                def norm_block(lo, hi):
                    # normalize oT cols [512c+lo, 512c+hi) for both heads.
                    # head h2 dims live at partitions 64*h2..+64 of outs[h2];
                    # the denominator is replicated on the OTHER 64 rows.
                    cols = slice(512 * c + lo, 512 * c + hi)
                    w = slice(lo, hi)
                    for h2 in range(2):
                        d = slice(64 * h2, 64 * h2 + 64)  # dims rows
                        e = slice(64 - 64 * h2, 128 - 64 * h2)  # denom rows
                        nc.vector.reciprocal(rsb[d, w], outs[h2][e, w])
                        nc.vector.tensor_mul(
                            oT_sb[d, pr, cols], outs[h2][d, w], rsb[d, w]
                        )

                def emit_av(item):
                    j, off, PT = item
                    for h2 in range(2):
                        nc.tensor.matmul(
                            outs[h2][:, off:512],
                            v_sb[:, j, pr, 128 * h2 : 128 * (h2 + 1)],
                            PT[:, 512 * h2 + off : 512 * (h2 + 1)],
                            start=(j == 0),
                            stop=(j == last),
                            # the per-block normalize reads columns whose
                            # accumulation is complete while later diagonal
                            # AVs still accumulate disjoint columns; the
                            # zero-region group check can't see that
                            skip_group_check=True,
                        )
                    if j >= 4 * c:
                        # col block [off, off+128) of outs is final after the
                        # diagonal AV for tile j = 4c + off/128 lands
                        b_end = off + 128
                        if b_end % norm_grain == 0:
                            lo = b_end - norm_grain
                            norm_block(lo, b_end)
                            if post_block is not None:
                                post_block(lo // 128, b_end // 128)

                def emit_qk(j):
                    off = P * (j - 4 * c) if j >= 4 * c else 0
                    S = psum.tile([P, 1024], FP, tag="s", name="S")
                    for h2 in range(2):
                        base = 64 * h2
                        nc.tensor.matmul(
                            S[:, 512 * h2 + off : 512 * (h2 + 1)],
                            kT_sb[base : base + 64, pr, P * j : P * (j + 1)],
                            qT_sb[base : base + 64, pr, 512 * c + off : 512 * (c + 1)],
                        )
                    return S, off

                ntiles = 4 * c + 4
                nxt = emit_qk(0)
                for j in range(ntiles):
                    S, off = nxt
                    PT = ptp.tile([P, 1024], BF, tag="pt", name="PT")
                    if off == 0:
                        nc.scalar.activation(PT, S, EXP, scale=1.0 / (DH * WS * WS))
                    else:
                        sv = S.rearrange("p (h q) -> p h q", h=2)[:, :, off:512]
                        pv_ = PT.rearrange("p (h q) -> p h q", h=2)[:, :, off:512]
                        nc.scalar.activation(pv_, sv, EXP, scale=1.0 / (DH * WS * WS))
                    # next tile's QK goes into the PE queue immediately so
                    # nothing sits between consecutive QKs and the exp
                    # stream stays saturated; AV/bg fill the exp window
                    if j + 1 < ntiles:
                        nxt = emit_qk(j + 1)
                    if j >= 4 * c:  # diagonal tile: triangle mask on GPSIMD
                        for h2 in range(2):
                            sl = slice(512 * h2 + off, 512 * h2 + off + P)
                            nc.gpsimd.tensor_mul(PT[:, sl], PT[:, sl], tri_sb)
                    if len(pend) >= av_lag:
                        emit_av(pend.pop(0))
                    pend.append((j, off, PT))
                    bg.drain_for_tile(t0 + j)
                while pend:
                    emit_av(pend.pop(0))

            # ---- program ----------------------------------------------
            def emit_program(skip_loads=False):
                if not skip_loads:
                    load_initial()
                ones_memset()

                bg = BG()
                # prologue foreground: q/k for (pr=0, half=0). pv(0) goes to
                # bg so its xT-slice DMA wait doesn't block the first QKs in
                # the in-order PE queue (av_lag covers the deferral).
                for g in (pq(wq_sb, qT_sb, 0, 0), pq(wk_sb, kT_sb, 0, 0)):
                    bg.add(g)
                bg.drain_all()
                bg.add(pv(0), 2)
                if not skip_loads:
                    load_rest()

                if upto == "proj":
                    for g in [pv(jt) for jt in range(1, KT)] + [
                        pq(m, d, pl, h)
                        for (m, d) in ((wq_sb, qT_sb), (wk_sb, kT_sb))
                        for pl in range(2)
                        for h in range(4)
                        if not (pl == 0 and h == 0)
                    ]:
                        bg.add(g)
                    bg.drain_all()
                    return

                # tile index by which each x8/xT chunk's DMA has landed
                # (load_rest order at ~1.1 us/tile pacing)
                EST_X8 = {0: 0, 1: 5, 2: 10, 3: 15}
                EST_XT = {0: 0, 1: 8, 2: 13, 3: 18}

                def dl_pv(jt):  # complete before the first AV that reads v[jt]
                    c_min = max(0, (jt - 3 + 3) // 4)  # ceil((jt-3)/4)
                    return CHUNK_START[(0, c_min)] + jt - 1

                def dl_pq(pl, h):  # complete before chunk (pl, h) starts
                    return CHUNK_START[(pl, h)] - 1

                def add_ops_for(c, tail):
                    def post_block(b_lo, b_hi):
                        for b in range(b_lo, b_hi):
                            bg.add(op(4 * c + b, tail))

                    return post_block

                # add the whole projection plan upfront, interleaved in need
                # order; est gates each item on its input chunk's DMA landing
                # and the deadline forces it in time for its first consumer
                adds = [(pv(1), dl_pv(1), 0), (pv(2), dl_pv(2), 0), (pv(3), dl_pv(3), 0)]
                for h in range(QC):
                    for pl in range(2):
                        if (pl, h) == (0, 0):
                            continue
                        adds.append((pq(wq_sb, qT_sb, pl, h), dl_pq(pl, h), EST_X8[h]))
                        adds.append((pq(wk_sb, kT_sb, pl, h), dl_pq(pl, h), EST_X8[h]))
                    if h >= 1:
                        for jt in range(4 * h, 4 * h + 4):
                            adds.append((pv(jt), dl_pv(jt), EST_XT[h]))
                adds.sort(key=lambda a: (a[1], a[2]))
                for item, dl, est in adds:
                    bg.add(item, dl, est)
                for ci, (pr, c) in enumerate(
                    [(pr, c) for c in range(QC) for pr in range(2)]
                ):
                    last_chunk = ci == 2 * QC - 1
                    if upto == "full":
                        post = None
                        if pr == 1:
                            # op quanta for this c become available per block
                            post = add_ops_for(c, last_chunk)
                        attn_chunk(
                            bg,
                            pr,
                            c,
                            norm_grain=128 if last_chunk else 256,
                            post_block=post,
                            av_lag=3 if ci == 0 else 2,
                        )
                    else:
                        bg.drain_all()
                bg.drain_all()

            if hw_loop:
                if loads_once:
                    load_initial()
                    load_rest()
                with tc.For_i(0, hw_loop, 1) as _i:
                    for _rep in range(repeat):
                        emit_program(skip_loads=True)
            else:
                for _rep in range(repeat):
                    emit_program()

    nc.compile()
    return nc


_NC = None


def _get_nc():
    global _NC
    if _NC is None:
        _NC = build_bass()
    return _NC


def _bf16(a):
    import ml_dtypes

    return np.asarray(a, dtype=np.float32).astype(ml_dtypes.bfloat16)


def _f8(a):
    import ml_dtypes

    return np.asarray(a, dtype=np.float32).astype(ml_dtypes.float8_e4m3fn)


def make_in_maps(x, w_q, w_kv, w_out):
    tri = np.triu(np.ones((P, P), dtype=np.float32))
    xTs = [np.ascontiguousarray(np.asarray(x[b], dtype=np.float32).T) for b in range(B)]
    w_q = np.asarray(w_q, dtype=np.float32)
    w_kv = np.asarray(w_kv, dtype=np.float32)
    w_out = np.asarray(w_out, dtype=np.float32)
    head_perm = np.r_[0:64, 128:192, 64:128, 192:256]  # h0,h2,h1,h3
    in_maps = []
    for c in range(NCORES):
        b, g = divmod(c, NCORES // B)
        wv_cols = w_kv[:, D + G * g : D + G * (g + 1)][:, head_perm]
        in_maps.append(
            {
                "xT": _bf16(xTs[b]),
                "xT8": _f8(xTs[b]),
                "wq": _f8(w_q[:, G * g : G * (g + 1)] * WS),
                "wk": _f8(w_kv[:, G * g : G * (g + 1)] * WS),
                "wv": _bf16(wv_cols),
                "wo": _bf16(w_out[G * g : G * (g + 1), :]),
                "tri": _bf16(tri),
            }
        )
    return in_maps


def combine_outputs(results, b_out):
    b_out = np.asarray(b_out, dtype=np.float32)
    y = np.zeros((B, N, D), dtype=np.float32)
    for c in range(NCORES):
        y[c // (NCORES // B)] += results[c]["y"]
    y += b_out
    return y


def kernel(x, w_q, w_kv, w_out, b_out):
    nc = _get_nc()
    in_maps = make_in_maps(x, w_q, w_kv, w_out)
    res = run_bass_kernel_spmd(nc, in_maps, core_ids=list(range(NCORES)))
    return combine_outputs(res.results, b_out)


# revision 32
# speedup vs baseline: 1.0370x; 1.0370x over previous
"""Trainium2 Bass kernel for nn_Attention (dense transformer block), v4.

v4 = v3 + structural pipeline work:
  - NOTE: fp8 for the v projection was tried and measured +3e-2 error: the
    projection output is itself a random-sign sum, so input quantization
    error does NOT average down over the contraction (error scales with
    sqrt(sum of squares) = the same sqrt(K) as the output). v stays bf16.
  - initial DMAs are split per feature plane / per 128-row x slice and
    ordered by need (wq/x8/wk planes feed the t-th matmul of the first q/k
    projection; xT slice jt gates pv(jt) alone), so the first matmul starts
    ~1.5 us after kernel start and attention chunk (0,0) by ~7 us.
  - normalize (recip+mul) is emitted per column block gated on the diagonal
    AV that finalizes that block, instead of one per-chunk burst: mid chunks
    use 256-col blocks, the final chunk 128-col blocks so the tail
    interleaves output-projection quanta with the last normalizes.
  - background matmul drain is deadline-driven with even smoothing over the
    remaining global tile count (v3's per-chunk formula front-loaded the
    drain and starved the last ~20 tiles, leaving ~300 ns/tile PE gaps).
  - initial DMAs are split per feature plane and spread across the SP and
    DVE queues in need order, so the first q/k projection matmul starts
    ~1.5 us after kernel start instead of ~4.6 us.
  - output-projection PSUM->SBUF copies alternate DVE/Pool mid-kernel and
    ACT/Pool in the tail (ACT is idle once the exp stream ends).

Reference computation (per batch b of 2, seq N=2048, dim D=1024, 16 heads x 64):
    q = (x @ w_q) / 64                      # source double-scales by d**-0.5
    k, v = split(x @ w_kv)
    per head: out_h = softmax(causal(q_h k_h^T)) v_h
    y = concat(out) @ w_out + b_out

Sharding (8 cores): core c -> batch b = c//4, head group g = c%4 (heads 4g..4g+3).
Each core computes its 4 heads end-to-end plus its partial output projection
(rows 256g..256g+256 of w_out); the host sums the 4 partials per batch and adds
b_out.

Carried from v3/v2:
  - fp8e4 DoubleRow for q/k/v projections (K=256/matmul); attention QK^T/AV
    and the output projection stay bf16 (fp8 there measured +2-3e-2 error).
  - AV lhsT per head is [v_h | c*ones] or [c*ones | v_h] (128 cols), so the
    softmax denominator comes out REPLICATED on 64 PSUM partitions; the
    normalization is DVE reciprocal + multiply with no PE broadcast.
  - the two heads' QK^T matmuls sit at lhsT base partitions 0/64 -> walrus
    derives row-group tile_position, so they overlap on the PE array.
  - engine assignment: ACT does exp; DVE does PSUM->SBUF copies + normalize;
    GPSIMD (Pool) does causal triangle masks, ones memset, some copies.
"""

import numpy as np

import concourse.bass as bass
import concourse.mybir as mybir
import concourse.tile as tile
from concourse import bacc
from concourse.bass_utils import run_bass_kernel_spmd

FP = mybir.dt.float32
BF = mybir.dt.bfloat16
F8 = mybir.dt.float8e4
DR = mybir.MatmulPerfMode.DoubleRow
EXP = mybir.ActivationFunctionType.Exp
WS = 32.0  # host prescale on w_q/w_k/w_v (exact power of 2)

B = 2
N = 2048  # sequence length
D = 1024  # model dim
NH = 4  # heads per core
DH = 64  # head dim
G = NH * DH  # 256 = per-core projection width
P = 128
DKT = D // P  # 8 feature k-tiles
PT2 = DKT // 2  # 4 paired fp8 feature planes
KT = N // P  # 16 sequence k-tiles
QC = N // 512  # 4 q chunks of 512
NCORES = 8

# global tile index at which chunk (pr, c) starts; chunk order is
# (0,0),(1,0),(0,1),(1,1),(0,2),(1,2),(0,3),(1,3)
CHUNK_START = {}
_t = 0
for _c in range(QC):
    for _pr in range(2):
        CHUNK_START[(_pr, _c)] = _t
        _t += 4 * _c + 4
TOTAL_TILES = _t  # 80


def build_bass(repeat=1, hw_loop=0, upto="full", loads_once=False):
    nc = bacc.Bacc("TRN2", target_bir_lowering=False, debug=False, num_devices=NCORES)

    xT = nc.dram_tensor("xT", [D, N], BF, kind="ExternalInput").ap()
    xT8 = nc.dram_tensor("xT8", [D, N], F8, kind="ExternalInput").ap()
    wq = nc.dram_tensor("wq", [D, G], F8, kind="ExternalInput").ap()
    wk = nc.dram_tensor("wk", [D, G], F8, kind="ExternalInput").ap()
    wv = nc.dram_tensor("wv", [D, G], BF, kind="ExternalInput").ap()  # cols h0,h2,h1,h3
    wo = nc.dram_tensor("wo", [G, D], BF, kind="ExternalInput").ap()
    tri = nc.dram_tensor("tri", [P, P], BF, kind="ExternalInput").ap()
    y = nc.dram_tensor("y", [N, D], FP, kind="ExternalOutput").ap()

    with tile.TileContext(nc) as tc:
        with (
            tc.tile_pool(name="const", bufs=1) as const,
            tc.tile_pool(name="ptp", bufs=4) as ptp,
            tc.tile_pool(name="ysb", bufs=4) as ysbp,
            tc.tile_pool(name="rsp", bufs=2) as rsp,
            tc.tile_pool(name="psum", bufs=2, space="PSUM") as psum,
        ):
            # fp8 paired layouts [P, t, ko, ...]: feature plane = 2t+ko
            wq_sb = const.tile([P, PT2, 2, G], F8)
            wk_sb = const.tile([P, PT2, 2, G], F8)
            xT8_sb = const.tile([P, PT2, 2, N], F8)
            wv_sb = const.tile([P, DKT, G], BF)
            xT_sb = const.tile([P, DKT, N], BF)
            wo_sb = const.tile([P, 2, D], BF)
            tri_sb = const.tile([P, P], BF)
            qT_sb = const.tile([P, 2, N], BF)
            kT_sb = const.tile([P, 2, N], BF)
            # v planes per (jtile, pr): [v_{2pr} | ones | ones | v_{2pr+1}]
            v_sb = const.tile([P, KT, 2, 256], BF)
            oT_sb = const.tile([P, 2, N], BF)

            xTr = xT.rearrange("(o p) m -> p o m", p=P)
            xT8r = xT8.rearrange("(t k p) n -> p t k n", p=P, k=2)
            wqr = wq.rearrange("(t k p) m -> p t k m", p=P, k=2)
            wkr = wk.rearrange("(t k p) m -> p t k m", p=P, k=2)

            def load_initial():
                # need order: plane t of {wq, x8 chunk0, wk} feeds the t-th
                # matmul of the first q/k projections; xT tile slices jt gate
                # pv(jt) individually. wv/tri ride the ACT queue (one issue
                # each, before the exp stream starts).
                # two issue queues in parallel: SP carries the q-side
                # criticals + x slices, ACT carries the k-side + wv/tri
                # (both are needed at the same time; issue overhead halves)
                for t in range(PT2):
                    nc.sync.dma_start(wq_sb[:, t], wqr[:, t])
                    nc.sync.dma_start(
                        xT8_sb[:, t, :, 0:512], xT8r[:, t, :, 0:512]
                    )
                    nc.scalar.dma_start(wk_sb[:, t], wkr[:, t])
                nc.scalar.dma_start(wv_sb, wv.rearrange("(o p) m -> p o m", p=P))
                nc.scalar.dma_start(tri_sb, tri)
                for jt in range(4):
                    nc.sync.dma_start(
                        xT_sb[:, :, P * jt : P * (jt + 1)],
                        xTr[:, :, P * jt : P * (jt + 1)],
                    )

            def load_rest():
                def x8_ch(ch):
                    nc.sync.dma_start(
                        xT8_sb[:, :, :, 512 * ch : 512 * (ch + 1)],
                        xT8r[:, :, :, 512 * ch : 512 * (ch + 1)],
                    )

                def xT_ch(ch):
                    nc.sync.dma_start(
                        xT_sb[:, :, 512 * ch : 512 * (ch + 1)],
                        xTr[:, :, 512 * ch : 512 * (ch + 1)],
                    )

                x8_ch(1)
                xT_ch(1)
                nc.sync.dma_start(wo_sb, wo.rearrange("(o p) m -> p o m", p=P))
                x8_ch(2)
                xT_ch(2)
                x8_ch(3)
                xT_ch(3)

            def ones_memset():
                nc.gpsimd.memset(v_sb[:, :, :, 64:192], 1.0)

            # ---- background quanta -------------------------------------
            def pq(w_sb, dst, pl, half):
                # dst[:, pl, 512*half:+512] = (w plane pl)^T @ xT cols
                def mms():
                    ps = psum.tile([P, 512], FP, tag="rb", name=f"ps_p{pl}h{half}")
                    lo = 512 * half
                    for t in range(PT2):
                        yield nc.tensor.matmul(
                            ps,
                            w_sb[:, t, :, P * pl : P * (pl + 1)],
                            xT8_sb[:, t, :, lo : lo + 512],
                            start=(t == 0),
                            stop=(t == PT2 - 1),
                            perf_mode=DR,
                        )
                    nc.vector.tensor_copy(dst[:, pl, lo : lo + 512], ps)

                return mms, PT2

            def pv(jt):
                # v rows 128*jt..+128, all 4 heads; wv host col order h0,h2,h1,h3
                def mms():
                    ps = psum.tile([P, G], FP, tag="rb", name="ps_v")
                    for j in range(DKT):
                        yield nc.tensor.matmul(
                            ps,
                            xT_sb[:, j, P * jt : P * (jt + 1)],
                            wv_sb[:, j, :],
                            start=(j == 0),
                            stop=(j == DKT - 1),
                        )
                    # one 4D copy: dst (pr, side) cols {0:64, 192:256},
                    # src head index a = pr + 2*side (host col order h0,h2,h1,h3)
                    psv = ps.rearrange("p (s pr d) -> p pr s d", s=2, pr=2)
                    dst = v_sb[:, jt].rearrange("p pr (s d) -> p pr s d", s=4)
                    nc.vector.tensor_copy(dst[:, :, 0:4:3, :], psv)

                return mms, DKT

            def op(i, tail=False):
                # y rows 128*i..+128
                def mms():
                    ysb = ysbp.tile([P, D], FP, name="ysb")
                    for n2 in range(2):
                        ps = psum.tile([P, 512], FP, tag="rb", name="ps_y")
                        for pr in range(2):
                            yield nc.tensor.matmul(
                                ps,
                                oT_sb[:, pr, P * i : P * (i + 1)],
                                wo_sb[:, pr, 512 * n2 : 512 * (n2 + 1)],
                                start=(pr == 0),
                                stop=(pr == 1),
                            )
                        dst = ysb[:, 512 * n2 : 512 * (n2 + 1)]
                        if tail and n2 == 0:  # ACT is idle once the exps end
                            nc.scalar.copy(dst, ps)
                        else:  # Pool can't read PSUM, so DVE otherwise
                            nc.vector.tensor_copy(dst, ps)
                        dmae = nc.scalar if tail and n2 == 1 else nc.sync
                        dmae.dma_start(
                            y[P * i : P * (i + 1), 512 * n2 : 512 * (n2 + 1)],
                            ysb[:, 512 * n2 : 512 * (n2 + 1)],
                        )

                return mms, 4

            class BG:
                """Deadline-driven drain of background matmul generators.

                Items carry (generator, mms_left, global-tile deadline or
                None, earliest-start tile). Per tile: drain everything
                overdue, plus an even smoothing share of the remaining work
                over the remaining tiles — but never an item before its
                earliest-start tile (its input DMA hasn't landed; emitting
                it would stall the in-order PE queue on the DMA).
                """

                def __init__(self):
                    self.items = []  # [gen, mms_left, deadline, est]
                    self.t_now = 0

                def add(self, fac_mms, deadline=None, est=0):
                    fac, mms = fac_mms
                    self.items.append([fac(), mms, deadline, est])

                def _drain_one(self):
                    # among eligible (est <= t_now): items due within 4 tiles
                    # get EDF priority; otherwise FIFO by add order (so
                    # far-deadline items don't starve deadline-less op work)
                    best = None
                    for it in self.items:
                        if it[3] > self.t_now:
                            continue
                        if it[2] is not None and it[2] <= self.t_now + 4 and (
                            best is None or best[2] is None or it[2] < best[2]
                        ):
                            best = it
                    if best is None:
                        for it in self.items:
                            if it[3] <= self.t_now:
                                best = it
                                break
                    if best is None:
                        return False
                    try:
                        next(best[0])
                        best[1] -= 1
                        if best[1] <= 0:
                            # let the generator run its epilogue (copies)
                            try:
                                next(best[0])
                            except StopIteration:
                                pass
                            self.items.remove(best)
                    except StopIteration:
                        self.items.remove(best)
                    return True

                def drain_n(self, n):
                    for _ in range(n):
                        if not self._drain_one():
                            return

                def drain_for_tile(self, t_global):
                    self.t_now = t_global
                    total = sum(it[1] for it in self.items)
                    if not total:
                        return
                    overdue = sum(
                        it[1]
                        for it in self.items
                        if it[2] is not None and it[2] <= t_global
                    )
                    remaining_tiles = max(1, TOTAL_TILES - t_global)
                    smooth = -(-total // remaining_tiles)  # ceil
                    self.drain_n(max(overdue, smooth))

                def drain_all(self):
                    self.t_now = 1 << 30
                    while self._drain_one():
                        pass

            # ---- attention ---------------------------------------------
            def attn_chunk(bg, pr, c, norm_grain=256, post_block=None, av_lag=1):
                # heads (2*pr, 2*pr+1); q columns 512*c..+512. av_lag defers
                # each AV by that many tiles behind its exp (PT pool holds 4),
                # decoupling the first chunk from the v-tile DMA stream.
                outs = [
                    psum.tile([P, 512], FP, tag="o", name=f"o{h2}") for h2 in range(2)
                ]
                rsb = rsp.tile([P, 512], FP, name="rsb")
                last = 4 * c + 3
                t0 = CHUNK_START[(pr, c)]
                pend = []

                def norm_block(lo, hi):
                    # normalize oT cols [512c+lo, 512c+hi) for both heads.
                    # head h2 dims live at partitions 64*h2..+64 of outs[h2];
                    # the denominator is replicated on the OTHER 64 rows.
                    cols = slice(512 * c + lo, 512 * c + hi)
                    w = slice(lo, hi)
                    for h2 in range(2):
                        d = slice(64 * h2, 64 * h2 + 64)  # dims rows
                        e = slice(64 - 64 * h2, 128 - 64 * h2)  # denom rows
                        nc.vector.reciprocal(rsb[d, w], outs[h2][e, w])
                        nc.vector.tensor_mul(
                            oT_sb[d, pr, cols], outs[h2][d, w], rsb[d, w]
                        )

                def emit_av(item):
                    j, off, PT = item
                    for h2 in range(2):
                        nc.tensor.matmul(
                            outs[h2][:, off:512],
                            v_sb[:, j, pr, 128 * h2 : 128 * (h2 + 1)],
                            PT[:, 512 * h2 + off : 512 * (h2 + 1)],
                            start=(j == 0),
                            stop=(j == last),
                            # the per-block normalize reads columns whose
                            # accumulation is complete while later diagonal
                            # AVs still accumulate disjoint columns; the
                            # zero-region group check can't see that
                            skip_group_check=True,
                        )
                    if j >= 4 * c:
                        # col block [off, off+128) of outs is final after the
                        # diagonal AV for tile j = 4c + off/128 lands
                        b_end = off + 128
                        if b_end % norm_grain == 0:
                            lo = b_end - norm_grain
                            norm_block(lo, b_end)
                            if post_block is not None:
                                post_block(lo // 128, b_end // 128)

                def emit_qk(j):
                    off = P * (j - 4 * c) if j >= 4 * c else 0
                    S = psum.tile([P, 1024], FP, tag="s", name="S")
                    for h2 in range(2):
                        base = 64 * h2
                        nc.tensor.matmul(
                            S[:, 512 * h2 + off : 512 * (h2 + 1)],
                            kT_sb[base : base + 64, pr, P * j : P * (j + 1)],
                            qT_sb[base : base + 64, pr, 512 * c + off : 512 * (c + 1)],
                        )
                    return S, off

                ntiles = 4 * c + 4
                nxt = emit_qk(0)
                for j in range(ntiles):
                    S, off = nxt
                    PT = ptp.tile([P, 1024], BF, tag="pt", name="PT")
                    if off == 0:
                        nc.scalar.activation(PT, S, EXP, scale=1.0 / (DH * WS * WS))
                    else:
                        sv = S.rearrange("p (h q) -> p h q", h=2)[:, :, off:512]
                        pv_ = PT.rearrange("p (h q) -> p h q", h=2)[:, :, off:512]
                        nc.scalar.activation(pv_, sv, EXP, scale=1.0 / (DH * WS * WS))
                    # next tile's QK goes into the PE queue immediately so
                    # nothing sits between consecutive QKs and the exp
                    # stream stays saturated; AV/bg fill the exp window
                    if j + 1 < ntiles:
                        nxt = emit_qk(j + 1)
                    if j >= 4 * c:  # diagonal tile: triangle mask on GPSIMD
                        for h2 in range(2):
                            sl = slice(512 * h2 + off, 512 * h2 + off + P)
                            nc.gpsimd.tensor_mul(PT[:, sl], PT[:, sl], tri_sb)
                    if len(pend) >= av_lag:
                        emit_av(pend.pop(0))
                    pend.append((j, off, PT))
                    bg.drain_for_tile(t0 + j)
                while pend:
                    emit_av(pend.pop(0))

            # ---- program ----------------------------------------------
            def emit_program(skip_loads=False):
                if not skip_loads:
                    load_initial()
                ones_memset()

                bg = BG()
                # prologue foreground: q/k for (pr=0, half=0). pv(0) goes to
                # bg so its xT-slice DMA wait doesn't block the first QKs in
                # the in-order PE queue (av_lag covers the deferral).
                for g in (pq(wq_sb, qT_sb, 0, 0), pq(wk_sb, kT_sb, 0, 0)):
                    bg.add(g)
                bg.drain_all()
                bg.add(pv(0), 2)
                if not skip_loads:
                    load_rest()

                if upto == "proj":
                    for g in [pv(jt) for jt in range(1, KT)] + [
                        pq(m, d, pl, h)
                        for (m, d) in ((wq_sb, qT_sb), (wk_sb, kT_sb))
                        for pl in range(2)
                        for h in range(4)
                        if not (pl == 0 and h == 0)
                    ]:
                        bg.add(g)
                    bg.drain_all()
                    return

                # tile index by which each x8/xT chunk's DMA has landed
                # (load_rest order at ~1.1 us/tile pacing)
                EST_X8 = {0: 0, 1: 5, 2: 10, 3: 15}
                EST_XT = {0: 0, 1: 8, 2: 13, 3: 18}

                def dl_pv(jt):  # complete before the first AV that reads v[jt]
                    c_min = max(0, (jt - 3 + 3) // 4)  # ceil((jt-3)/4)
                    return CHUNK_START[(0, c_min)] + jt - 1

                def dl_pq(pl, h):  # complete before chunk (pl, h) starts
                    return CHUNK_START[(pl, h)] - 1

                def add_ops_for(c, tail):
                    def post_block(b_lo, b_hi):
                        for b in range(b_lo, b_hi):
                            bg.add(op(4 * c + b, tail))

                    return post_block

                # add the whole projection plan upfront, interleaved in need
                # order; est gates each item on its input chunk's DMA landing
                # and the deadline forces it in time for its first consumer
                adds = [(pv(1), dl_pv(1), 0), (pv(2), dl_pv(2), 0), (pv(3), dl_pv(3), 0)]
                for h in range(QC):
                    for pl in range(2):
                        if (pl, h) == (0, 0):
                            continue
                        adds.append((pq(wq_sb, qT_sb, pl, h), dl_pq(pl, h), EST_X8[h]))
                        adds.append((pq(wk_sb, kT_sb, pl, h), dl_pq(pl, h), EST_X8[h]))
                    if h >= 1:
                        for jt in range(4 * h, 4 * h + 4):
                            adds.append((pv(jt), dl_pv(jt), EST_XT[h]))
                adds.sort(key=lambda a: (a[1], a[2]))
                for item, dl, est in adds:
                    bg.add(item, dl, est)
                for ci, (pr, c) in enumerate(
                    [(pr, c) for c in range(QC) for pr in range(2)]
                ):
                    last_chunk = ci == 2 * QC - 1
                    if upto == "full":
                        post = None
                        if pr == 1:
                            # op quanta for this c become available per block
                            post = add_ops_for(c, last_chunk)
                        attn_chunk(
                            bg,
                            pr,
                            c,
                            norm_grain=128 if last_chunk else 256,
                            post_block=post,
                            av_lag=3 if ci == 0 else 1,
                        )
                    else:
                        bg.drain_all()
                bg.drain_all()

            if hw_loop:
                if loads_once:
                    load_initial()
                    load_rest()
                with tc.For_i(0, hw_loop, 1) as _i:
                    for _rep in range(repeat):
                        emit_program(skip_loads=True)
            else:
                for _rep in range(repeat):
                    emit_program()

    nc.compile()
    return nc


_NC = None


def _get_nc():
    global _NC
    if _NC is None:
        _NC = build_bass()
    return _NC


def _bf16(a):
    import ml_dtypes

    return np.asarray(a, dtype=np.float32).astype(ml_dtypes.bfloat16)


def _f8(a):
    import ml_dtypes

    return np.asarray(a, dtype=np.float32).astype(ml_dtypes.float8_e4m3fn)


def make_in_maps(x, w_q, w_kv, w_out):
    tri = np.triu(np.ones((P, P), dtype=np.float32))
    xTs = [np.ascontiguousarray(np.asarray(x[b], dtype=np.float32).T) for b in range(B)]
    w_q = np.asarray(w_q, dtype=np.float32)
    w_kv = np.asarray(w_kv, dtype=np.float32)
    w_out = np.asarray(w_out, dtype=np.float32)
    head_perm = np.r_[0:64, 128:192, 64:128, 192:256]  # h0,h2,h1,h3
    in_maps = []
    for c in range(NCORES):
        b, g = divmod(c, NCORES // B)
        wv_cols = w_kv[:, D + G * g : D + G * (g + 1)][:, head_perm]
        in_maps.append(
            {
                "xT": _bf16(xTs[b]),
                "xT8": _f8(xTs[b]),
                "wq": _f8(w_q[:, G * g : G * (g + 1)] * WS),
                "wk": _f8(w_kv[:, G * g : G * (g + 1)] * WS),
                "wv": _bf16(wv_cols),
                "wo": _bf16(w_out[G * g : G * (g + 1), :]),
                "tri": _bf16(tri),
            }
        )
    return in_maps


def combine_outputs(results, b_out):
    b_out = np.asarray(b_out, dtype=np.float32)
    y = np.zeros((B, N, D), dtype=np.float32)
    for c in range(NCORES):
        y[c // (NCORES // B)] += results[c]["y"]
    y += b_out
    return y


def kernel(x, w_q, w_kv, w_out, b_out):
    nc = _get_nc()
    in_maps = make_in_maps(x, w_q, w_kv, w_out)
    res = run_bass_kernel_spmd(nc, in_maps, core_ids=list(range(NCORES)))
    return combine_outputs(res.results, b_out)


# revision 33
# speedup vs baseline: 1.3961x; 1.3463x over previous
"""Trainium2 Bass kernel for nn_Attention (dense transformer block), v4.

v4 = v3 + structural pipeline work:
  - NOTE: fp8 for the v projection was tried and measured +3e-2 error: the
    projection output is itself a random-sign sum, so input quantization
    error does NOT average down over the contraction (error scales with
    sqrt(sum of squares) = the same sqrt(K) as the output). v stays bf16.
  - initial DMAs are split per feature plane / per 128-row x slice and
    ordered by need (wq/x8/wk planes feed the t-th matmul of the first q/k
    projection; xT slice jt gates pv(jt) alone), so the first matmul starts
    ~1.5 us after kernel start and attention chunk (0,0) by ~7 us.
  - normalize (recip+mul) is emitted per column block gated on the diagonal
    AV that finalizes that block, instead of one per-chunk burst: mid chunks
    use 256-col blocks, the final chunk 128-col blocks so the tail
    interleaves output-projection quanta with the last normalizes. The AV
    matmuls set skip_group_check (the zero-region group check can't see
    that the read columns' accumulation is complete).
  - each tile's QK is emitted one tile ahead (S pool is double-buffered), so
    nothing sits between consecutive QKs in the in-order PE queue and the
    ACT exp stream stays saturated; AV runs av_lag tiles behind its exp
    (3 in the first chunk to decouple from the v-tile DMA stream).
  - background matmul drain is deadline+earliest-start driven with even
    smoothing over the remaining global tile count (v3's per-chunk formula
    front-loaded the drain and starved the last ~20 tiles at ~300 ns/tile,
    and eager drain of far-deadline items stalled the in-order PE queue on
    input DMAs that hadn't landed).
  - ysb ring deepened to 4 and the tail output DMAs split SP/ACT so the
    final output-projection tiles aren't serialized behind y-DMA semaphores;
    tail PSUM->SBUF copies go to ACT (idle once the exp stream ends).

Reference computation (per batch b of 2, seq N=2048, dim D=1024, 16 heads x 64):
    q = (x @ w_q) / 64                      # source double-scales by d**-0.5
    k, v = split(x @ w_kv)
    per head: out_h = softmax(causal(q_h k_h^T)) v_h
    y = concat(out) @ w_out + b_out

Sharding (8 cores): core c -> batch b = c//4, head group g = c%4 (heads 4g..4g+3).
Each core computes its 4 heads end-to-end plus its partial output projection
(rows 256g..256g+256 of w_out); the host sums the 4 partials per batch and adds
b_out.

Carried from v3/v2:
  - fp8e4 DoubleRow for q/k/v projections (K=256/matmul); attention QK^T/AV
    and the output projection stay bf16 (fp8 there measured +2-3e-2 error).
  - AV lhsT per head is [v_h | c*ones] or [c*ones | v_h] (128 cols), so the
    softmax denominator comes out REPLICATED on 64 PSUM partitions; the
    normalization is DVE reciprocal + multiply with no PE broadcast.
  - the two heads' QK^T matmuls sit at lhsT base partitions 0/64 -> walrus
    derives row-group tile_position, so they overlap on the PE array.
  - engine assignment: ACT does exp; DVE does PSUM->SBUF copies + normalize;
    GPSIMD (Pool) does causal triangle masks, ones memset, some copies.
"""

import numpy as np

import concourse.bass as bass
import concourse.mybir as mybir
import concourse.tile as tile
from concourse import bacc
from concourse.bass_utils import run_bass_kernel_spmd

FP = mybir.dt.float32
BF = mybir.dt.bfloat16
F8 = mybir.dt.float8e4
DR = mybir.MatmulPerfMode.DoubleRow
EXP = mybir.ActivationFunctionType.Exp
WS = 32.0  # host prescale on w_q/w_k/w_v (exact power of 2)

B = 2
N = 2048  # sequence length
D = 1024  # model dim
NH = 4  # heads per core
DH = 64  # head dim
G = NH * DH  # 256 = per-core projection width
P = 128
DKT = D // P  # 8 feature k-tiles
PT2 = DKT // 2  # 4 paired fp8 feature planes
KT = N // P  # 16 sequence k-tiles
QC = N // 512  # 4 q chunks of 512
NCORES = 8

# global tile index at which chunk (pr, c) starts; chunk order is
# (0,0),(1,0),(0,1),(1,1),(0,2),(1,2),(0,3),(1,3)
CHUNK_START = {}
_t = 0
for _c in range(QC):
    for _pr in range(2):
        CHUNK_START[(_pr, _c)] = _t
        _t += 4 * _c + 4
TOTAL_TILES = _t  # 80


def build_bass(repeat=1, hw_loop=0, upto="full", loads_once=False):
    nc = bacc.Bacc("TRN2", target_bir_lowering=False, debug=False, num_devices=NCORES)

    xT = nc.dram_tensor("xT", [D, N], BF, kind="ExternalInput").ap()
    xT8 = nc.dram_tensor("xT8", [D, N], F8, kind="ExternalInput").ap()
    wq = nc.dram_tensor("wq", [D, G], F8, kind="ExternalInput").ap()
    wk = nc.dram_tensor("wk", [D, G], F8, kind="ExternalInput").ap()
    wv = nc.dram_tensor("wv", [D, G], BF, kind="ExternalInput").ap()  # cols h0,h2,h1,h3
    wo = nc.dram_tensor("wo", [G, D], BF, kind="ExternalInput").ap()
    tri = nc.dram_tensor("tri", [P, P], BF, kind="ExternalInput").ap()
    y = nc.dram_tensor("y", [N, D], FP, kind="ExternalOutput").ap()

    with tile.TileContext(nc) as tc:
        with (
            tc.tile_pool(name="const", bufs=1) as const,
            tc.tile_pool(name="ptp", bufs=4) as ptp,
            tc.tile_pool(name="ysb", bufs=4) as ysbp,
            tc.tile_pool(name="rsp", bufs=2) as rsp,
            tc.tile_pool(name="psum", bufs=2, space="PSUM") as psum,
        ):
            # fp8 paired layouts [P, t, ko, ...]: feature plane = 2t+ko
            wq_sb = const.tile([P, PT2, 2, G], F8)
            wk_sb = const.tile([P, PT2, 2, G], F8)
            xT8_sb = const.tile([P, PT2, 2, N], F8)
            wv_sb = const.tile([P, DKT, G], BF)
            xT_sb = const.tile([P, DKT, N], BF)
            wo_sb = const.tile([P, 2, D], BF)
            tri_sb = const.tile([P, P], BF)
            qT_sb = const.tile([P, 2, N], BF)
            kT_sb = const.tile([P, 2, N], BF)
            # v planes per (jtile, pr): [v_{2pr} | ones | ones | v_{2pr+1}]
            v_sb = const.tile([P, KT, 2, 256], BF)
            oT_sb = const.tile([P, 2, N], BF)

            xTr = xT.rearrange("(o p) m -> p o m", p=P)
            xT8r = xT8.rearrange("(t k p) n -> p t k n", p=P, k=2)
            wqr = wq.rearrange("(t k p) m -> p t k m", p=P, k=2)
            wkr = wk.rearrange("(t k p) m -> p t k m", p=P, k=2)

            def load_initial():
                # need order: plane t of {wq, x8 chunk0, wk} feeds the t-th
                # matmul of the first q/k projections; xT tile slices jt gate
                # pv(jt) individually. wv/tri ride the ACT queue (one issue
                # each, before the exp stream starts).
                # two issue queues in parallel: SP carries the q-side
                # criticals + x slices, ACT carries the k-side + wv/tri
                # (both are needed at the same time; issue overhead halves)
                for t in range(PT2):
                    nc.sync.dma_start(wq_sb[:, t], wqr[:, t])
                    nc.sync.dma_start(
                        xT8_sb[:, t, :, 0:512], xT8r[:, t, :, 0:512]
                    )
                    nc.scalar.dma_start(wk_sb[:, t], wkr[:, t])
                nc.scalar.dma_start(wv_sb, wv.rearrange("(o p) m -> p o m", p=P))
                nc.scalar.dma_start(tri_sb, tri)
                for jt in range(4):
                    nc.sync.dma_start(
                        xT_sb[:, :, P * jt : P * (jt + 1)],
                        xTr[:, :, P * jt : P * (jt + 1)],
                    )

            def load_rest():
                def x8_ch(ch):
                    nc.sync.dma_start(
                        xT8_sb[:, :, :, 512 * ch : 512 * (ch + 1)],
                        xT8r[:, :, :, 512 * ch : 512 * (ch + 1)],
                    )

                def xT_ch(ch):
                    nc.sync.dma_start(
                        xT_sb[:, :, 512 * ch : 512 * (ch + 1)],
                        xTr[:, :, 512 * ch : 512 * (ch + 1)],
                    )

                x8_ch(1)
                xT_ch(1)
                nc.sync.dma_start(wo_sb, wo.rearrange("(o p) m -> p o m", p=P))
                x8_ch(2)
                xT_ch(2)
                x8_ch(3)
                xT_ch(3)

            def ones_memset():
                nc.gpsimd.memset(v_sb[:, :, :, 64:192], 1.0)

            # ---- background quanta -------------------------------------
            def pq(w_sb, dst, pl, half):
                # dst[:, pl, 512*half:+512] = (w plane pl)^T @ xT cols
                def mms():
                    ps = psum.tile([P, 512], FP, tag="rb", name=f"ps_p{pl}h{half}")
                    lo = 512 * half
                    for t in range(PT2):
                        yield nc.tensor.matmul(
                            ps,
                            w_sb[:, t, :, P * pl : P * (pl + 1)],
                            xT8_sb[:, t, :, lo : lo + 512],
                            start=(t == 0),
                            stop=(t == PT2 - 1),
                            perf_mode=DR,
                        )
                    nc.vector.tensor_copy(dst[:, pl, lo : lo + 512], ps)

                return mms, PT2

            def pv(jt):
                # v rows 128*jt..+128, all 4 heads; wv host col order h0,h2,h1,h3
                def mms():
                    ps = psum.tile([P, G], FP, tag="rb", name="ps_v")
                    for j in range(DKT):
                        yield nc.tensor.matmul(
                            ps,
                            xT_sb[:, j, P * jt : P * (jt + 1)],
                            wv_sb[:, j, :],
                            start=(j == 0),
                            stop=(j == DKT - 1),
                        )
                    # one 4D copy: dst (pr, side) cols {0:64, 192:256},
                    # src head index a = pr + 2*side (host col order h0,h2,h1,h3)
                    psv = ps.rearrange("p (s pr d) -> p pr s d", s=2, pr=2)
                    dst = v_sb[:, jt].rearrange("p pr (s d) -> p pr s d", s=4)
                    nc.vector.tensor_copy(dst[:, :, 0:4:3, :], psv)

                return mms, DKT

            def op(i, tail=False):
                # y rows 128*i..+128
                def mms():
                    ysb = ysbp.tile([P, D], FP, name="ysb")
                    for n2 in range(2):
                        ps = psum.tile([P, 512], FP, tag="rb", name="ps_y")
                        for pr in range(2):
                            yield nc.tensor.matmul(
                                ps,
                                oT_sb[:, pr, P * i : P * (i + 1)],
                                wo_sb[:, pr, 512 * n2 : 512 * (n2 + 1)],
                                start=(pr == 0),
                                stop=(pr == 1),
                            )
                        dst = ysb[:, 512 * n2 : 512 * (n2 + 1)]
                        if tail and n2 == 0:  # ACT is idle once the exps end
                            nc.scalar.copy(dst, ps)
                        else:  # Pool can't read PSUM, so DVE otherwise
                            nc.vector.tensor_copy(dst, ps)
                        dmae = nc.scalar if tail and n2 == 1 else nc.sync
                        dmae.dma_start(
                            y[P * i : P * (i + 1), 512 * n2 : 512 * (n2 + 1)],
                            ysb[:, 512 * n2 : 512 * (n2 + 1)],
                        )

                return mms, 4

            class BG:
                """Deadline-driven drain of background matmul generators.

                Items carry (generator, mms_left, global-tile deadline or
                None, earliest-start tile). Per tile: drain everything
                overdue, plus an even smoothing share of the remaining work
                over the remaining tiles — but never an item before its
                earliest-start tile (its input DMA hasn't landed; emitting
                it would stall the in-order PE queue on the DMA).
                """

                def __init__(self):
                    self.items = []  # [gen, mms_left, deadline, est]
                    self.t_now = 0

                def add(self, fac_mms, deadline=None, est=0):
                    fac, mms = fac_mms
                    self.items.append([fac(), mms, deadline, est])

                def _drain_one(self):
                    # among eligible (est <= t_now): items due within 4 tiles
                    # get EDF priority; otherwise FIFO by add order (so
                    # far-deadline items don't starve deadline-less op work)
                    best = None
                    for it in self.items:
                        if it[3] > self.t_now:
                            continue
                        if it[2] is not None and it[2] <= self.t_now + 4 and (
                            best is None or best[2] is None or it[2] < best[2]
                        ):
                            best = it
                    if best is None:
                        for it in self.items:
                            if it[3] <= self.t_now:
                                best = it
                                break
                    if best is None:
                        return False
                    try:
                        next(best[0])
                        best[1] -= 1
                        if best[1] <= 0:
                            # let the generator run its epilogue (copies)
                            try:
                                next(best[0])
                            except StopIteration:
                                pass
                            self.items.remove(best)
                    except StopIteration:
                        self.items.remove(best)
                    return True

                def drain_n(self, n):
                    for _ in range(n):
                        if not self._drain_one():
                            return

                def drain_for_tile(self, t_global):
                    self.t_now = t_global
                    total = sum(it[1] for it in self.items)
                    if not total:
                        return
                    overdue = sum(
                        it[1]
                        for it in self.items
                        if it[2] is not None and it[2] <= t_global
                    )
                    remaining_tiles = max(1, TOTAL_TILES - t_global)
                    smooth = -(-total // remaining_tiles)  # ceil
                    self.drain_n(max(overdue, smooth))

                def drain_all(self):
                    self.t_now = 1 << 30
                    while self._drain_one():
                        pass

            # ---- attention ---------------------------------------------
            def attn_chunk(bg, pr, c, norm_grain=256, post_block=None, av_lag=1):
                # heads (2*pr, 2*pr+1); q columns 512*c..+512. av_lag defers
                # each AV by that many tiles behind its exp (PT pool holds 4),
                # decoupling the first chunk from the v-tile DMA stream.
                outs = [
                    psum.tile([P, 512], FP, tag="o", name=f"o{h2}") for h2 in range(2)
                ]
                rsb = rsp.tile([P, 512], FP, name="rsb")
                last = 4 * c + 3
                t0 = CHUNK_START[(pr, c)]
                pend = []

                def norm_block(lo, hi):
                    # normalize oT cols [512c+lo, 512c+hi) for both heads.
                    # head h2 dims live at partitions 64*h2..+64 of outs[h2];
                    # the denominator is replicated on the OTHER 64 rows.
                    cols = slice(512 * c + lo, 512 * c + hi)
                    w = slice(lo, hi)
                    for h2 in range(2):
                        d = slice(64 * h2, 64 * h2 + 64)  # dims rows
                        e = slice(64 - 64 * h2, 128 - 64 * h2)  # denom rows
                        nc.vector.reciprocal(rsb[d, w], outs[h2][e, w])
                        nc.vector.tensor_mul(
                            oT_sb[d, pr, cols], outs[h2][d, w], rsb[d, w]
                        )

                def emit_av(item):
                    j, off, PT = item
                    for h2 in range(2):
                        nc.tensor.matmul(
                            outs[h2][:, off:512],
                            v_sb[:, j, pr, 128 * h2 : 128 * (h2 + 1)],
                            PT[:, 512 * h2 + off : 512 * (h2 + 1)],
                            start=(j == 0),
                            stop=(j == last),
                            # the per-block normalize reads columns whose
                            # accumulation is complete while later diagonal
                            # AVs still accumulate disjoint columns; the
                            # zero-region group check can't see that
                            skip_group_check=True,
                        )
                    if j >= 4 * c:
                        # col block [off, off+128) of outs is final after the
                        # diagonal AV for tile j = 4c + off/128 lands
                        b_end = off + 128
                        if b_end % norm_grain == 0:
                            lo = b_end - norm_grain
                            norm_block(lo, b_end)
                            if post_block is not None:
                                post_block(lo // 128, b_end // 128)

                def emit_qk(j):
                    off = P * (j - 4 * c) if j >= 4 * c else 0
                    S = psum.tile([P, 1024], FP, tag="s", name="S")
                    for h2 in range(2):
                        base = 64 * h2
                        nc.tensor.matmul(
                            S[:, 512 * h2 + off : 512 * (h2 + 1)],
                            kT_sb[base : base + 64, pr, P * j : P * (j + 1)],
                            qT_sb[base : base + 64, pr, 512 * c + off : 512 * (c + 1)],
                        )
                    return S, off

                ntiles = 4 * c + 4
                nxt = emit_qk(0)
                for j in range(ntiles):
                    S, off = nxt
                    PT = ptp.tile([P, 1024], BF, tag="pt", name="PT")
                    if off == 0:
                        nc.scalar.activation(PT, S, EXP, scale=1.0 / (DH * WS * WS))
                    else:
                        sv = S.rearrange("p (h q) -> p h q", h=2)[:, :, off:512]
                        pv_ = PT.rearrange("p (h q) -> p h q", h=2)[:, :, off:512]
                        nc.scalar.activation(pv_, sv, EXP, scale=1.0 / (DH * WS * WS))
                    # next tile's QK goes into the PE queue immediately so
                    # nothing sits between consecutive QKs and the exp
                    # stream stays saturated; AV/bg fill the exp window
                    if j + 1 < ntiles:
                        nxt = emit_qk(j + 1)
                    if j >= 4 * c:  # diagonal tile: triangle mask on GPSIMD
                        for h2 in range(2):
                            sl = slice(512 * h2 + off, 512 * h2 + off + P)
                            nc.gpsimd.tensor_mul(PT[:, sl], PT[:, sl], tri_sb)
                    if len(pend) >= av_lag:
                        emit_av(pend.pop(0))
                    pend.append((j, off, PT))
                    bg.drain_for_tile(t0 + j)
                while pend:
                    emit_av(pend.pop(0))

            # ---- program ----------------------------------------------
            def emit_program(skip_loads=False):
                if not skip_loads:
                    load_initial()
                ones_memset()

                bg = BG()
                # prologue foreground: q/k for (pr=0, half=0). pv(0) goes to
                # bg so its xT-slice DMA wait doesn't block the first QKs in
                # the in-order PE queue (av_lag covers the deferral).
                for g in (pq(wq_sb, qT_sb, 0, 0), pq(wk_sb, kT_sb, 0, 0)):
                    bg.add(g)
                bg.drain_all()
                bg.add(pv(0), 2)
                if not skip_loads:
                    load_rest()

                if upto == "proj":
                    for g in [pv(jt) for jt in range(1, KT)] + [
                        pq(m, d, pl, h)
                        for (m, d) in ((wq_sb, qT_sb), (wk_sb, kT_sb))
                        for pl in range(2)
                        for h in range(4)
                        if not (pl == 0 and h == 0)
                    ]:
                        bg.add(g)
                    bg.drain_all()
                    return

                # tile index by which each x8/xT chunk's DMA has landed
                # (load_rest order at ~1.1 us/tile pacing)
                EST_X8 = {0: 0, 1: 5, 2: 10, 3: 15}
                EST_XT = {0: 0, 1: 8, 2: 13, 3: 18}

                def dl_pv(jt):  # complete before the first AV that reads v[jt]
                    c_min = max(0, (jt - 3 + 3) // 4)  # ceil((jt-3)/4)
                    return CHUNK_START[(0, c_min)] + jt - 1

                def dl_pq(pl, h):  # complete before chunk (pl, h) starts
                    return CHUNK_START[(pl, h)] - 1

                def add_ops_for(c, tail):
                    def post_block(b_lo, b_hi):
                        for b in range(b_lo, b_hi):
                            bg.add(op(4 * c + b, tail))

                    return post_block

                # add the whole projection plan upfront, interleaved in need
                # order; est gates each item on its input chunk's DMA landing
                # and the deadline forces it in time for its first consumer
                adds = [(pv(1), dl_pv(1), 0), (pv(2), dl_pv(2), 0), (pv(3), dl_pv(3), 0)]
                for h in range(QC):
                    for pl in range(2):
                        if (pl, h) == (0, 0):
                            continue
                        adds.append((pq(wq_sb, qT_sb, pl, h), dl_pq(pl, h), EST_X8[h]))
                        adds.append((pq(wk_sb, kT_sb, pl, h), dl_pq(pl, h), EST_X8[h]))
                    if h >= 1:
                        for jt in range(4 * h, 4 * h + 4):
                            adds.append((pv(jt), dl_pv(jt), EST_XT[h]))
                adds.sort(key=lambda a: (a[1], a[2]))
                for item, dl, est in adds:
                    bg.add(item, dl, est)
                for ci, (pr, c) in enumerate(
                    [(pr, c) for c in range(QC) for pr in range(2)]
                ):
                    last_chunk = ci == 2 * QC - 1
                    if upto == "full":
                        post = None
                        if pr == 1:
                            # op quanta for this c become available per block
                            post = add_ops_for(c, last_chunk)
                        attn_chunk(
                            bg,
                            pr,
                            c,
                            norm_grain=128 if last_chunk else 256,
                            post_block=post,
                            av_lag=3 if ci == 0 else 1,
                        )
                    else:
                        bg.drain_all()
                bg.drain_all()

            if hw_loop:
                if loads_once:
                    load_initial()
                    load_rest()
                with tc.For_i(0, hw_loop, 1) as _i:
                    for _rep in range(repeat):
                        emit_program(skip_loads=True)
            else:
                for _rep in range(repeat):
                    emit_program()

    nc.compile()
    return nc


_NC = None


def _get_nc():
    global _NC
    if _NC is None:
        _NC = build_bass()
    return _NC


def _bf16(a):
    import ml_dtypes

    return np.asarray(a, dtype=np.float32).astype(ml_dtypes.bfloat16)


def _f8(a):
    import ml_dtypes

    return np.asarray(a, dtype=np.float32).astype(ml_dtypes.float8_e4m3fn)


def make_in_maps(x, w_q, w_kv, w_out):
    tri = np.triu(np.ones((P, P), dtype=np.float32))
    xTs = [np.ascontiguousarray(np.asarray(x[b], dtype=np.float32).T) for b in range(B)]
    w_q = np.asarray(w_q, dtype=np.float32)
    w_kv = np.asarray(w_kv, dtype=np.float32)
    w_out = np.asarray(w_out, dtype=np.float32)
    head_perm = np.r_[0:64, 128:192, 64:128, 192:256]  # h0,h2,h1,h3
    in_maps = []
    for c in range(NCORES):
        b, g = divmod(c, NCORES // B)
        wv_cols = w_kv[:, D + G * g : D + G * (g + 1)][:, head_perm]
        in_maps.append(
            {
                "xT": _bf16(xTs[b]),
                "xT8": _f8(xTs[b]),
                "wq": _f8(w_q[:, G * g : G * (g + 1)] * WS),
                "wk": _f8(w_kv[:, G * g : G * (g + 1)] * WS),
                "wv": _bf16(wv_cols),
                "wo": _bf16(w_out[G * g : G * (g + 1), :]),
                "tri": _bf16(tri),
            }
        )
    return in_maps


def combine_outputs(results, b_out):
    b_out = np.asarray(b_out, dtype=np.float32)
    y = np.zeros((B, N, D), dtype=np.float32)
    for c in range(NCORES):
        y[c // (NCORES // B)] += results[c]["y"]
    y += b_out
    return y


def kernel(x, w_q, w_kv, w_out, b_out):
    nc = _get_nc()
    in_maps = make_in_maps(x, w_q, w_kv, w_out)
    res = run_bass_kernel_spmd(nc, in_maps, core_ids=list(range(NCORES)))
    return combine_outputs(res.results, b_out)
